# revision 1
# baseline (speedup 1.0000x reference)
"""CRTN middle_l query construction as a pure-DMA Bass kernel on 8 TRN2 cores.

Math (from the reference):
    query_base = concat([neighbor_mem[-1], wise_inputs], axis=0)   # (256, B, H)
    query[i, j] = query_base[i + j + 1]                            # (S, S, B, H)

For fixed i, query[i] = query_base[i+1 : i+129] is one contiguous 8 MB slab —
the whole problem is memory-bound replication: 16 MB of source fanned out to
1 GiB of output, bounded by per-core HBM/DMA write bandwidth.

Sharding: data-parallel over the output axis i (S=128 -> 16 rows per core).
Core k stages query_base rows [16k+1, 16k+144) (143 rows, 9.4 MB) in SBUF,
then writes 16 contiguous 8 MB output slabs.

Layout (the part that matters for speed): each 64 KB row is split into 8
chunks of 8 KB; chunk id c = 8*row + t lives at SBUF partition c % 128,
column c // 128 (9 columns, 72 KB/partition).  Each output row is then
covered by <= 9 rectangular SBUF->DRAM DMAs whose partition start AND count
are always multiples of 8, seven of them exactly 128 partitions.  Measured
on TRN2: DMAs with partition counts not divisible by 8 fall off the HWDGE
fast path and run ~5x slower (~77 GB/s vs ~400+ GB/s); this chunked layout
keeps every transfer on the fast path (~360 us/core vs 1.9 ms for the naive
row-per-partition version).
"""

import numpy as np

import concourse.bacc as bacc
import concourse.bass as bass
import concourse.mybir as mybir
import concourse.tile as tile
from concourse.bass_utils import run_bass_kernel_spmd

# Problem shape (hardcoded; harness contract forbids reading spec.json here).
NEI_LEN = 128
S = 128
B = 16
H = 1024
N_CORES = 8
ROWS_PER_CORE = S // N_CORES          # 16 output rows (values of i) per core
IN_ROWS = ROWS_PER_CORE + S - 1       # 143 query_base rows staged per core
ROW_ELEMS = B * H                     # 16384 f32 = 64 KB per query_base row
T = 8                                 # chunks per row
CH = ROW_ELEMS // T                   # 2048 f32 = 8 KB per chunk
N_CHUNKS = T * IN_ROWS                # 1144
N_COLS = (N_CHUNKS + 127) // 128      # 9 SBUF columns
WIN = T * S                           # 1024 chunks per output row

# Timing side-channel for test harnesses (exec_time_ns when a profile ran).
LAST_EXEC_NS = None

_nc_cache = None


def _build_nc(repeats: int = 1) -> bass.Bass:
    # Bacc (not raw Bass): its compile() pass splits multi-sem waits into
    # event-semaphore chains — the walrus codegen rejects instructions with
    # more than one sync wait ("Too many sync wait commands").
    #
    # repeats > 1 unrolls the body N times (idempotent — same bytes written
    # each round); bench harnesses use the K-vs-1 slope of wall-clock exec
    # time to extract per-iteration HW time through the axon tunnel, which
    # has no NTFF profiling hook.
    nc = bacc.Bacc("TRN2", target_bir_lowering=False, debug=False)
    qb = nc.dram_tensor(
        "qb", [IN_ROWS, ROW_ELEMS], mybir.dt.float32, kind="ExternalInput"
    )
    out = nc.dram_tensor(
        "out", [ROWS_PER_CORE, WIN, CH], mybir.dt.float32, kind="ExternalOutput"
    )
    qb_chunks = qb.ap().rearrange("r (t o) -> (r t) o", t=T)  # (1144, 2048)
    with tile.TileContext(nc) as tc:
        with tc.tile_pool(name="stage", bufs=min(repeats, 2)) as pool:
            for _ in range(repeats):
                buf = pool.tile([128, N_COLS * CH], mybir.dt.float32)
                for c in range(N_COLS):
                    lo, hi = 128 * c, min(128 * (c + 1), N_CHUNKS)
                    nc.sync.dma_start(
                        out=buf[0 : hi - lo, c * CH : (c + 1) * CH],
                        in_=qb_chunks[lo:hi, :],
                    )
                for m in range(ROWS_PER_CORE):
                    # Output row m = chunk window [8m, 8m + 1024); intersect
                    # with each SBUF column -> rects with partition start and
                    # count always divisible by 8 (HWDGE fast path).
                    w_lo = T * m
                    for c in range(N_COLS):
                        lo = max(128 * c, w_lo)
                        hi = min(128 * (c + 1), w_lo + WIN)
                        if lo >= hi:
                            continue
                        p0 = lo - 128 * c
                        nc.sync.dma_start(
                            out=out[m, lo - w_lo : hi - w_lo, :],
                            in_=buf[p0 : p0 + hi - lo, c * CH : (c + 1) * CH],
                        )
    nc.compile()
    return nc


def kernel(neighbor_mem: np.ndarray, wise_inputs: np.ndarray) -> np.ndarray:
    global _nc_cache, LAST_EXEC_NS
    assert neighbor_mem.shape == (13, NEI_LEN, B, H), neighbor_mem.shape
    assert wise_inputs.shape == (S, B, H), wise_inputs.shape

    qb_full = np.concatenate(
        [
            np.asarray(neighbor_mem[-1], dtype=np.float32).reshape(NEI_LEN, ROW_ELEMS),
            np.asarray(wise_inputs, dtype=np.float32).reshape(S, ROW_ELEMS),
        ],
        axis=0,
    )  # (256, 16384)

    in_maps = [
        {"qb": qb_full[ROWS_PER_CORE * k + 1 : ROWS_PER_CORE * k + 1 + IN_ROWS]}
        for k in range(N_CORES)
    ]

    if _nc_cache is None:
        _nc_cache = _build_nc()

    res = run_bass_kernel_spmd(_nc_cache, in_maps, core_ids=list(range(N_CORES)))
    LAST_EXEC_NS = res.exec_time_ns

    # out[m, k, :] with k = 8j + t is exactly row-major (S, B, H) per m.
    out = np.concatenate(
        [r["out"].reshape(ROWS_PER_CORE, S, B, H) for r in res.results], axis=0
    )
    return out



# revision 2
# speedup vs baseline: 1.2379x; 1.2379x over previous
"""CRTN middle_l query construction as a pure-DMA Bass kernel on 8 TRN2 cores.

Math (from the reference):
    query_base = concat([neighbor_mem[-1], wise_inputs], axis=0)   # (256, B, H)
    query[i, j] = query_base[i + j + 1]                            # (S, S, B, H)

For fixed i, query[i] = query_base[i+1 : i+129] is one contiguous 8 MB slab —
the whole problem is memory-bound replication: 16 MB of source fanned out to
1 GiB of output, bounded by per-core HBM/DMA write bandwidth (~360 GB/s →
~400 us/core floor for the 144 MB/core of DMA traffic).

Sharding: data-parallel over the output axis i (S=128 -> 16 rows per core).
Core k stages query_base rows [16k+1, 16k+144) (143 rows, 9.4 MB) in SBUF,
then writes 16 contiguous 8 MB output slabs.

Layout: each 64 KB row is split into 8 chunks of 8 KB; chunk id c = 8*row + t
lives at SBUF partition c % 128, column c // 128 (9 columns). Output row m
covers the chunk window [8m, 8m+1024); columns 1..7 of the window are full
128-partition rectangles for EVERY m, and the DRAM address of out[m] is
linear in (partition, column, element). That lets the 7 middle columns of
each output row go out as ONE 7 MiB three-dim-AP DMA; only the window edges
(column 0 and column 8 partials, 1 MiB total per row) need separate
transfers. Per core: 3 staging DMAs + 16 big + 31 edge = 50 DMAs (vs 156 for
the per-column version), every transfer with partition start/count divisible
by 8 (the HWDGE fast path — misaligned partition counts measured ~5x slower)
and >= 64 KB. Big and edge DMAs alternate between the two HWDGE rings
(nc.sync = SP, nc.scalar = ACT) so per-DMA completion latency overlaps.
"""

import numpy as np

import concourse.bacc as bacc
import concourse.bass as bass
import concourse.mybir as mybir
import concourse.tile as tile
from concourse.bass_utils import run_bass_kernel_spmd

# Problem shape (hardcoded; harness contract forbids reading spec.json here).
NEI_LEN = 128
S = 128
B = 16
H = 1024
N_CORES = 8
ROWS_PER_CORE = S // N_CORES          # 16 output rows (values of i) per core
IN_ROWS = ROWS_PER_CORE + S - 1       # 143 query_base rows staged per core
ROW_ELEMS = B * H                     # 16384 f32 = 64 KB per query_base row
T = 8                                 # chunks per row
CH = ROW_ELEMS // T                   # 2048 f32 = 8 KB per chunk
N_CHUNKS = T * IN_ROWS                # 1144
WIN = T * S                           # 1024 chunks per output row

# Timing side-channel for test harnesses (exec_time_ns when a profile ran).
LAST_EXEC_NS = None

_nc_cache = None


def _build_nc(repeats: int = 1) -> bass.Bass:
    # Bacc (not raw Bass): its compile() pass splits multi-sem waits into
    # event-semaphore chains — the walrus codegen rejects instructions with
    # more than one sync wait ("Too many sync wait commands").
    #
    # repeats > 1 unrolls the body N times (idempotent — same bytes written
    # each round); bench harnesses use the K-vs-1 slope of wall-clock exec
    # time to extract per-iteration HW time through the axon tunnel, which
    # has no NTFF profiling hook.
    nc = bacc.Bacc("TRN2", target_bir_lowering=False, debug=False)
    qb = nc.dram_tensor(
        "qb", [IN_ROWS, ROW_ELEMS], mybir.dt.float32, kind="ExternalInput"
    )
    out = nc.dram_tensor(
        "out", [ROWS_PER_CORE, WIN, CH], mybir.dt.float32, kind="ExternalOutput"
    )
    qb_chunks = qb.ap().rearrange("r (t o) -> (r t) o", t=T)  # (1144, 2048)
    with tile.TileContext(nc) as tc:
        with tc.tile_pool(name="stage", bufs=min(repeats, 2)) as pool:
            for _ in range(repeats):
                # A[p, 7j:7(j+1)... column j] = chunk 128*(j+1) + p  (cols 1..7)
                A = pool.tile([128, 7 * CH], mybir.dt.float32)
                # Bt col 0 = chunks 0..128, col 1 = chunks 1024..1144
                Bt = pool.tile([128, 2 * CH], mybir.dt.float32)
                nc.sync.dma_start(
                    out=A[0:128, :].rearrange("p (j e) -> p j e", j=7),
                    in_=qb_chunks[128 : 128 + 896, :].rearrange(
                        "(j p) e -> p j e", j=7, p=128
                    ),
                )
                nc.scalar.dma_start(out=Bt[0:128, 0:CH], in_=qb_chunks[0:128, :])
                nc.scalar.dma_start(
                    out=Bt[0:120, CH : 2 * CH], in_=qb_chunks[1024:1144, :]
                )
                engines = [nc.sync, nc.scalar]
                for m in range(ROWS_PER_CORE):
                    eng = engines[m % 2]
                    oth = engines[(m + 1) % 2]
                    # Window columns 1..7: out[m, 128c-8m+p, :] = chunk 128c+p
                    # = A[p, c-1] — one 7 MiB DMA, APs 3-dim on both sides.
                    eng.dma_start(
                        out=out[m, 128 - 8 * m : 1024 - 8 * m].rearrange(
                            "(c p) e -> p c e", c=7, p=128
                        ),
                        in_=A[0:128, :].rearrange("p (c e) -> p c e", c=7),
                    )
                    # Window column 0 partial: chunks 8m..128.
                    oth.dma_start(
                        out=out[m, 0 : 128 - 8 * m],
                        in_=Bt[8 * m : 128, 0:CH],
                    )
                    if m > 0:
                        # Window column 8 partial: chunks 1024..1024+8m.
                        oth.dma_start(
                            out=out[m, WIN - 8 * m : WIN],
                            in_=Bt[0 : 8 * m, CH : 2 * CH],
                        )
    nc.compile()
    return nc


def kernel(neighbor_mem: np.ndarray, wise_inputs: np.ndarray) -> np.ndarray:
    global _nc_cache, LAST_EXEC_NS
    assert neighbor_mem.shape == (13, NEI_LEN, B, H), neighbor_mem.shape
    assert wise_inputs.shape == (S, B, H), wise_inputs.shape

    qb_full = np.concatenate(
        [
            np.asarray(neighbor_mem[-1], dtype=np.float32).reshape(NEI_LEN, ROW_ELEMS),
            np.asarray(wise_inputs, dtype=np.float32).reshape(S, ROW_ELEMS),
        ],
        axis=0,
    )  # (256, 16384)

    in_maps = [
        {"qb": qb_full[ROWS_PER_CORE * k + 1 : ROWS_PER_CORE * k + 1 + IN_ROWS]}
        for k in range(N_CORES)
    ]

    if _nc_cache is None:
        _nc_cache = _build_nc()

    res = run_bass_kernel_spmd(_nc_cache, in_maps, core_ids=list(range(N_CORES)))
    LAST_EXEC_NS = res.exec_time_ns

    # out[m, k, :] with k = 8j + t is exactly row-major (S, B, H) per m.
    out = np.concatenate(
        [r["out"].reshape(ROWS_PER_CORE, S, B, H) for r in res.results], axis=0
    )
    return out
